# revision 1
# baseline (speedup 1.0000x reference)
"""Trainium2 Bass kernel for 3x3 valid conv (C_in=8, C_out=8, H=W=2048).

Strategy (spatial H-sharding across 8 cores):
  - Host splits x rows into 8 slabs of 256 output rows (+2 halo input rows)
    and packs each slab (fp16) into the exact SBUF layout the TensorE wants:
        xp[(ci, r), b, w] = slab[ci, h0(b) + r, w]
    for 19 row-blocks b (h0 = 14*b, last block 242), r = 0..15. Halo rows are
    duplicated host-side so every device load is a plain contiguous slice.
  - Three lhsT weight matrices (one per kw) of shape [K=128, M=112]:
        K = (ci, r), M = (co, j) with j = 0..13
        lhsT[kw][ci*16 + r, co*14 + j] = W[co, ci, r - j, kw]  (0 <= r-j <= 2)
  - Device per core: for each group of 4 blocks, one DMA loads
    [128, 4*2048] fp16; per block and per 512-wide column tile, 3
    accumulating matmuls (kw = 0,1,2; rhs shifted along the free axis)
    produce [112, 512] fp32 PSUM = out[(co, j), w]; PSUM tiles are copied
    (DVE/ACT alternating, fp32->fp16) into a [112, 4*2046] tile and stored
    with one DMA into op[(co, j), b, w]. Host scatters op back to
    (C, 2046, 2046) fp32.
"""

import numpy as np

import concourse.bass as bass
import concourse.mybir as mybir
import concourse.tile as tile
from concourse import bacc
from concourse.bass_utils import run_bass_kernel_spmd

# ---- problem geometry (hardcoded) ----
C = 8
H = 2048
W = 2048
KH = KW = 3
H_OUT = H - KH + 1   # 2046
W_OUT = W - KW + 1   # 2046
N_CORES = 8

ROWS_PER_CORE = 256          # output rows computed per core (core 7: 254 valid)
IN_ROWS = ROWS_PER_CORE + 2  # 258 input rows per core slab

J = 14                       # output rows per block
R = J + 2                    # 16 input rows per block
K = C * R                    # 128 contraction partitions
M = C * J                    # 112 output partitions
NBLK = 19                    # blocks per core
BLOCK_STARTS = [J * b for b in range(NBLK - 1)] + [ROWS_PER_CORE - J]
# h0(b) = 14*b for b<18, h0(18) = 242 (overlap-recompute tail)

COL_TILES = [(0, 512), (512, 512), (1024, 512), (1536, 510)]

IN_DT = mybir.dt.float16     # on-wire activation dtype
IN_NP = np.float16
OUT_DT = mybir.dt.float16    # on-wire output dtype (host upcasts)
OUT_NP = np.float16

GRP = 2                      # blocks per store DMA group
LOAD_GRP = 1                 # blocks per load DMA (divides into GRP groups)
Y_BUFS = 6
O_BUFS = 4


def build_nc(repeat: int = 1, mode: str = "full", grp: int = GRP,
             load_grp: int = LOAD_GRP, y_bufs: int = Y_BUFS, o_bufs: int = O_BUFS,
             load_eng: str = "pool"):
    do_mm = mode in ("full", "nocopy")
    do_copy = mode in ("full",)
    do_dma = mode in ("full", "nocopy", "dma")
    groups = [list(range(s, min(s + grp, NBLK))) for s in range(0, NBLK, grp)]
    nc = bacc.Bacc(
        "TRN2",
        target_bir_lowering=False,
        debug=False,
        num_devices=N_CORES,
    )
    xp = nc.dram_tensor("xp", [K, NBLK, W], IN_DT, kind="ExternalInput").ap()
    wts = nc.dram_tensor("wts", [KW, K, M], IN_DT, kind="ExternalInput").ap()
    op = nc.dram_tensor("op", [M, NBLK, W_OUT], OUT_DT, kind="ExternalOutput").ap()

    with tile.TileContext(nc) as tc:
        with (
            tc.tile_pool(name="wpool", bufs=1) as wpool,
            tc.tile_pool(name="ypool", bufs=y_bufs) as ypool,
            tc.tile_pool(name="opool", bufs=o_bufs) as opool,
            tc.tile_pool(name="pspool", bufs=8, space="PSUM") as pspool,
        ):
            wsb = wpool.tile([K, KW * M], IN_DT)
            for kw in range(KW):
                nc.sync.dma_start(wsb[:, kw * M:(kw + 1) * M], wts[kw])

            for rep_i in range(repeat):
                for blocks in groups:
                    g = len(blocks)
                    b0 = blocks[0]
                    # y is loaded in load_grp-block chunks for finer PE overlap
                    ys = []
                    for s in range(0, g, load_grp):
                        gl = min(load_grp, g - s)
                        yt = ypool.tile([K, gl * W], IN_DT, name="y", tag="y",
                                        padded_shape=[K, load_grp * W])
                        if do_dma:
                            if load_eng == "pool":
                                eng = nc.gpsimd
                            elif load_eng == "sp":
                                eng = nc.sync
                            else:  # alternate
                                eng = nc.gpsimd if (b0 + s) % (2 * load_grp) else nc.sync
                            eng.dma_start(yt[:], xp[:, b0 + s:b0 + s + gl, :])
                        ys.append(yt)

                    o = opool.tile([M, g * W_OUT], OUT_DT, name="o", tag="o",
                                   padded_shape=[M, grp * W_OUT])
                    for bi in range(g):
                        pss = []
                        for ti in range(len(COL_TILES)):
                            ps = pspool.tile([M, 512], mybir.dt.float32,
                                             name=f"ps{ti}", tag="ps")
                            pss.append(ps)
                        if do_mm:
                            y = ys[bi // load_grp]
                            yb = bi % load_grp
                            # kw-outer: consecutive MMs share the stationary side
                            for kw in range(KW):
                                for ti, (w0, n) in enumerate(COL_TILES):
                                    c0 = yb * W + w0 + kw
                                    nc.tensor.matmul(
                                        pss[ti][:, :n],
                                        lhsT=wsb[:, kw * M:(kw + 1) * M],
                                        rhs=y[:, c0:c0 + n],
                                        start=(kw == 0),
                                        stop=(kw == KW - 1),
                                    )
                        if do_copy:
                            for ti, (w0, n) in enumerate(COL_TILES):
                                dst = o[:, bi * W_OUT + w0:bi * W_OUT + w0 + n]
                                if ti % 2 == 0:
                                    nc.vector.tensor_copy(dst, pss[ti][:, :n])
                                else:
                                    nc.scalar.copy(dst, pss[ti][:, :n])
                    if not do_copy and do_dma:
                        # ablation modes: cheap writer so Tile allocates o
                        nc.vector.memset(o[:, :8], 0.0)
                    if do_dma:
                        nc.sync.dma_start(op[:, b0:b0 + g, :], o[:])

    nc.compile()
    return nc


def build_weight_lhst(weight: np.ndarray) -> np.ndarray:
    """weight: (C_out, C_in, 3, 3) fp32 -> (3, K, M) IN_NP."""
    wl = np.zeros((KW, K, M), np.float32)
    ci = np.arange(C)
    for kw in range(KW):
        for co in range(C):
            for j in range(J):
                for kh in range(KH):
                    r = j + kh
                    wl[kw, ci * R + r, co * J + j] = weight[co, :, kh, kw]
    return wl.astype(IN_NP)


def pack_core_input(slab: np.ndarray) -> np.ndarray:
    """slab: (C, IN_ROWS, W) fp16 -> xp (K, NBLK, W) fp16."""
    s0, s1, s2 = slab.strides
    # b = 0..17 uniform stride J; b = 18 special (h0 = 242)
    v = np.lib.stride_tricks.as_strided(
        slab, shape=(C, R, NBLK - 1, W), strides=(s0, s1, J * s1, s2)
    )
    xp = np.empty((C, R, NBLK, W), slab.dtype)
    xp[:, :, :NBLK - 1, :] = v
    xp[:, :, NBLK - 1, :] = slab[:, BLOCK_STARTS[-1]:BLOCK_STARTS[-1] + R, :]
    return xp.reshape(K, NBLK, W)


def unpack_core_output(op: np.ndarray) -> np.ndarray:
    """op: (M, NBLK, W_OUT) -> (C, ROWS_PER_CORE, W_OUT) float32."""
    op = op.reshape(C, J, NBLK, W_OUT)
    res = np.empty((C, ROWS_PER_CORE, W_OUT), np.float32)
    res[:, BLOCK_STARTS[-1]:, :] = op[:, :, NBLK - 1, :].astype(np.float32)
    res[:, :J * (NBLK - 1), :] = (
        op[:, :, :NBLK - 1, :].transpose(0, 2, 1, 3).reshape(C, J * (NBLK - 1), W_OUT)
    )
    return res


def shard_inputs(x: np.ndarray, weight: np.ndarray):
    xc = np.ascontiguousarray(x).astype(IN_NP)
    wl = build_weight_lhst(weight)
    in_maps = []
    for i in range(N_CORES):
        lo = i * ROWS_PER_CORE
        hi = min(lo + IN_ROWS, H)
        if hi - lo == IN_ROWS:
            slab = xc[:, lo:hi, :]
        else:
            slab = np.zeros((C, IN_ROWS, W), IN_NP)
            slab[:, :hi - lo, :] = xc[:, lo:hi, :]
        in_maps.append({"xp": pack_core_input(slab), "wts": wl})
    return in_maps


def unshard_output(results) -> np.ndarray:
    parts = []
    for i in range(N_CORES):
        rows = ROWS_PER_CORE if i < N_CORES - 1 else H_OUT - (N_CORES - 1) * ROWS_PER_CORE
        parts.append(unpack_core_output(results[i]["op"])[:, :rows, :])
    return np.concatenate(parts, axis=1)


_NC_CACHE = None


def _get_nc():
    global _NC_CACHE
    if _NC_CACHE is None:
        _NC_CACHE = build_nc()
    return _NC_CACHE


def run(inputs: dict, **spmd_kwargs):
    """Run the conv on 8 NeuronCores. Returns (full_output, BassKernelResults)."""
    in_maps = shard_inputs(np.asarray(inputs["x"]), np.asarray(inputs["weight"]))
    nc = _get_nc()
    res = run_bass_kernel_spmd(nc, in_maps, core_ids=list(range(N_CORES)), **spmd_kwargs)
    return unshard_output(res.results).astype(np.float32), res


def kernel(**inputs) -> np.ndarray:
    out, _ = run(inputs)
    return out



# revision 5
# speedup vs baseline: 1.0126x; 1.0126x over previous
"""Trainium2 Bass kernel for 3x3 valid conv (C_in=8, C_out=8, H=W=2048).

Strategy (spatial H-sharding across 8 cores):
  - Host splits x rows into 8 slabs of 256 output rows (+2 halo input rows)
    and packs each slab (fp16) into the exact SBUF layout the TensorE wants:
        xp[(ci, r), b, w] = slab[ci, h0(b) + r, w]
    for 19 row-blocks b (h0 = 14*b, last block 242), r = 0..15. Halo rows are
    duplicated host-side so every device load is a plain contiguous slice.
  - Three lhsT weight matrices (one per kw) of shape [K=128, M=112]:
        K = (ci, r), M = (co, j) with j = 0..13
        lhsT[kw][ci*16 + r, co*14 + j] = W[co, ci, r - j, kw]  (0 <= r-j <= 2)
  - Device per core: for each group of 4 blocks, one DMA loads
    [128, 4*2048] fp16; per block and per 512-wide column tile, 3
    accumulating matmuls (kw = 0,1,2; rhs shifted along the free axis)
    produce [112, 512] fp32 PSUM = out[(co, j), w]; PSUM tiles are copied
    (DVE/ACT alternating, fp32->fp16) into a [112, 4*2046] tile and stored
    with one DMA into op[(co, j), b, w]. Host scatters op back to
    (C, 2046, 2046) fp32.
"""

import numpy as np

import concourse.bass as bass
import concourse.mybir as mybir
import concourse.tile as tile
from concourse import bacc
from concourse.bass_utils import run_bass_kernel_spmd

# ---- problem geometry (hardcoded) ----
C = 8
H = 2048
W = 2048
KH = KW = 3
H_OUT = H - KH + 1   # 2046
W_OUT = W - KW + 1   # 2046
N_CORES = 8

ROWS_PER_CORE = 256          # output rows computed per core (core 7: 254 valid)
IN_ROWS = ROWS_PER_CORE + 2  # 258 input rows per core slab

J = 14                       # output rows per block
R = J + 2                    # 16 input rows per block
K = C * R                    # 128 contraction partitions
M = C * J                    # 112 output partitions
NBLK = 19                    # blocks per core
BLOCK_STARTS = [J * b for b in range(NBLK - 1)] + [ROWS_PER_CORE - J]
# h0(b) = 14*b for b<18, h0(18) = 242 (overlap-recompute tail)

COL_TILES = [(0, 512), (512, 512), (1024, 512), (1536, 510)]

import ml_dtypes

IN_DT = mybir.dt.float8e3    # on-wire activation dtype (e3m4: 1B, rel err 2^-5)
IN_NP = ml_dtypes.float8_e3m4
W_DT = mybir.dt.float16      # weights stay fp16 (exact); mixed-dtype matmul
W_NP = np.float16
OUT_DT = mybir.dt.float16    # on-wire output dtype (host upcasts)
OUT_NP = np.float16

GRP = 2                      # blocks per store DMA group
LOAD_GRP = 1                 # blocks per load DMA (divides into GRP groups)
Y_BUFS = 6
O_BUFS = 4


def build_nc(repeat: int = 1, mode: str = "full", grp: int = GRP,
             load_grp: int = LOAD_GRP, y_bufs: int = Y_BUFS, o_bufs: int = O_BUFS,
             load_eng: str = "pool"):
    do_mm = mode in ("full", "nocopy")
    do_copy = mode in ("full",)
    do_dma = mode in ("full", "nocopy", "dma")
    groups = [list(range(s, min(s + grp, NBLK))) for s in range(0, NBLK, grp)]
    nc = bacc.Bacc(
        "TRN2",
        target_bir_lowering=False,
        debug=False,
        num_devices=N_CORES,
    )
    xp = nc.dram_tensor("xp", [K, NBLK, W], IN_DT, kind="ExternalInput").ap()
    wts = nc.dram_tensor("wts", [KW, K, M], W_DT, kind="ExternalInput").ap()
    op = nc.dram_tensor("op", [M, NBLK, W_OUT], OUT_DT, kind="ExternalOutput").ap()

    with tile.TileContext(nc) as tc:
        with (
            tc.tile_pool(name="wpool", bufs=1) as wpool,
            tc.tile_pool(name="ypool", bufs=y_bufs) as ypool,
            tc.tile_pool(name="opool", bufs=o_bufs) as opool,
            tc.tile_pool(name="pspool", bufs=8, space="PSUM") as pspool,
        ):
            wsb = wpool.tile([K, KW * M], W_DT)
            for kw in range(KW):
                nc.sync.dma_start(wsb[:, kw * M:(kw + 1) * M], wts[kw])

            for rep_i in range(repeat):
                for blocks in groups:
                    g = len(blocks)
                    b0 = blocks[0]
                    # y is loaded in load_grp-block chunks for finer PE overlap
                    ys = []
                    for s in range(0, g, load_grp):
                        gl = min(load_grp, g - s)
                        yt = ypool.tile([K, gl * W], IN_DT, name="y", tag="y",
                                        padded_shape=[K, load_grp * W])
                        if do_dma:
                            if load_eng == "pool":
                                eng = nc.gpsimd
                            elif load_eng == "sp":
                                eng = nc.sync
                            else:  # alternate
                                eng = nc.gpsimd if (b0 + s) % (2 * load_grp) else nc.sync
                            eng.dma_start(yt[:], xp[:, b0 + s:b0 + s + gl, :])
                        ys.append(yt)

                    o = opool.tile([M, g * W_OUT], OUT_DT, name="o", tag="o",
                                   padded_shape=[M, grp * W_OUT])
                    for bi in range(g):
                        pss = []
                        for ti in range(len(COL_TILES)):
                            ps = pspool.tile([M, 512], mybir.dt.float32,
                                             name=f"ps{ti}", tag="ps")
                            pss.append(ps)
                        if do_mm:
                            y = ys[bi // load_grp]
                            yb = bi % load_grp
                            # kw-outer: consecutive MMs share the stationary side
                            for kw in range(KW):
                                for ti, (w0, n) in enumerate(COL_TILES):
                                    c0 = yb * W + w0 + kw
                                    nc.tensor.matmul(
                                        pss[ti][:, :n],
                                        lhsT=wsb[:, kw * M:(kw + 1) * M],
                                        rhs=y[:, c0:c0 + n],
                                        start=(kw == 0),
                                        stop=(kw == KW - 1),
                                    )
                        if do_copy:
                            for ti, (w0, n) in enumerate(COL_TILES):
                                dst = o[:, bi * W_OUT + w0:bi * W_OUT + w0 + n]
                                if ti % 2 == 0:
                                    nc.vector.tensor_copy(dst, pss[ti][:, :n])
                                else:
                                    nc.scalar.copy(dst, pss[ti][:, :n])
                    if not do_copy and do_dma:
                        # ablation modes: cheap writer so Tile allocates o
                        nc.vector.memset(o[:, :8], 0.0)
                    if do_dma:
                        nc.sync.dma_start(op[:, b0:b0 + g, :], o[:])

    nc.compile()
    return nc


def build_weight_lhst(weight: np.ndarray) -> np.ndarray:
    """weight: (C_out, C_in, 3, 3) fp32 -> (3, K, M) IN_NP."""
    wl = np.zeros((KW, K, M), np.float32)
    ci = np.arange(C)
    for kw in range(KW):
        for co in range(C):
            for j in range(J):
                for kh in range(KH):
                    r = j + kh
                    wl[kw, ci * R + r, co * J + j] = weight[co, :, kh, kw]
    return wl.astype(W_NP)


def pack_core_input(slab: np.ndarray) -> np.ndarray:
    """slab: (C, IN_ROWS, W) fp16 -> xp (K, NBLK, W) fp16."""
    s0, s1, s2 = slab.strides
    # b = 0..17 uniform stride J; b = 18 special (h0 = 242)
    v = np.lib.stride_tricks.as_strided(
        slab, shape=(C, R, NBLK - 1, W), strides=(s0, s1, J * s1, s2)
    )
    xp = np.empty((C, R, NBLK, W), slab.dtype)
    xp[:, :, :NBLK - 1, :] = v
    xp[:, :, NBLK - 1, :] = slab[:, BLOCK_STARTS[-1]:BLOCK_STARTS[-1] + R, :]
    return xp.reshape(K, NBLK, W)


def unpack_core_output(op: np.ndarray) -> np.ndarray:
    """op: (M, NBLK, W_OUT) -> (C, ROWS_PER_CORE, W_OUT) float32."""
    op = op.reshape(C, J, NBLK, W_OUT)
    res = np.empty((C, ROWS_PER_CORE, W_OUT), np.float32)
    res[:, BLOCK_STARTS[-1]:, :] = op[:, :, NBLK - 1, :].astype(np.float32)
    res[:, :J * (NBLK - 1), :] = (
        op[:, :, :NBLK - 1, :].transpose(0, 2, 1, 3).reshape(C, J * (NBLK - 1), W_OUT)
    )
    return res


def shard_inputs(x: np.ndarray, weight: np.ndarray):
    xc = np.ascontiguousarray(x).astype(IN_NP)
    wl = build_weight_lhst(weight)
    in_maps = []
    for i in range(N_CORES):
        lo = i * ROWS_PER_CORE
        hi = min(lo + IN_ROWS, H)
        if hi - lo == IN_ROWS:
            slab = xc[:, lo:hi, :]
        else:
            slab = np.zeros((C, IN_ROWS, W), IN_NP)
            slab[:, :hi - lo, :] = xc[:, lo:hi, :]
        in_maps.append({"xp": pack_core_input(slab), "wts": wl})
    return in_maps


def unshard_output(results) -> np.ndarray:
    parts = []
    for i in range(N_CORES):
        rows = ROWS_PER_CORE if i < N_CORES - 1 else H_OUT - (N_CORES - 1) * ROWS_PER_CORE
        parts.append(unpack_core_output(results[i]["op"])[:, :rows, :])
    return np.concatenate(parts, axis=1)


_NC_CACHE = None


def _get_nc():
    global _NC_CACHE
    if _NC_CACHE is None:
        _NC_CACHE = build_nc()
    return _NC_CACHE


def run(inputs: dict, **spmd_kwargs):
    """Run the conv on 8 NeuronCores. Returns (full_output, BassKernelResults)."""
    in_maps = shard_inputs(np.asarray(inputs["x"]), np.asarray(inputs["weight"]))
    nc = _get_nc()
    res = run_bass_kernel_spmd(nc, in_maps, core_ids=list(range(N_CORES)), **spmd_kwargs)
    return unshard_output(res.results).astype(np.float32), res


def kernel(**inputs) -> np.ndarray:
    out, _ = run(inputs)
    return out



# revision 7
# speedup vs baseline: 1.0163x; 1.0037x over previous
"""Trainium2 Bass kernel for 3x3 valid conv (C_in=8, C_out=8, H=W=2048).

Strategy (spatial H-sharding across 8 cores):
  - Host splits x rows into 8 slabs of 256 output rows (+2 halo input rows)
    and packs each slab into the exact SBUF layout the TensorE wants:
        xp[(ci, r), b, w] = slab[ci, h0(b) + r, w]
    for 19 row-blocks b (h0 = 14*b, last block 242), r = 0..15. Halo rows are
    duplicated host-side so every device load is a plain contiguous slice.
  - On-wire dtypes: activations go over HBM as float8e3 (e3m4, 1 B/elem;
    measured end-to-end rel err 1.45e-2 vs the 2e-2 budget), weights stay
    fp16 and the TensorE runs the matmul with mixed operand dtypes
    (fp16 lhsT x fp8e3 rhs -> fp32 PSUM, verified bit-accurate on HW).
    This halves input HBM traffic vs fp16; outputs remain fp16.
  - Three lhsT weight matrices (one per kw) of shape [K=128, M=112]:
        K = (ci, r), M = (co, j) with j = 0..13
        lhsT[kw][ci*16 + r, co*14 + j] = W[co, ci, r - j, kw]  (0 <= r-j <= 2)
  - Device per core: per block, one DMA loads [128, 2048] fp8 (the first
    block in two column halves so the PE starts earlier); per 512-wide
    column tile, 3 accumulating matmuls (kw = 0,1,2; rhs shifted along the
    free axis) produce [112, 512] fp32 PSUM = out[(co, j), w]; PSUM tiles
    are copied (DVE/ACT alternating, fp32->fp16) into a [112, 2046] tile
    and stored per block into op[(co, j), b, w]. Host scatters op back to
    (C, 2046, 2046) fp32.

  The schedule is PE-bound: 228 ldweights+matmul pairs/core at ~244 ns
  each (512 cols * 0.417 ns + ~30 ns fixed per-pair overhead) ~= 56 us,
  with DMA (13.7 MB/core at ~360 GB/s ~= 38 us) fully overlapped.
"""

import numpy as np

import concourse.bass as bass
import concourse.mybir as mybir
import concourse.tile as tile
from concourse import bacc
from concourse.bass_utils import run_bass_kernel_spmd

# ---- problem geometry (hardcoded) ----
C = 8
H = 2048
W = 2048
KH = KW = 3
H_OUT = H - KH + 1   # 2046
W_OUT = W - KW + 1   # 2046
N_CORES = 8

ROWS_PER_CORE = 256          # output rows computed per core (core 7: 254 valid)
IN_ROWS = ROWS_PER_CORE + 2  # 258 input rows per core slab

J = 14                       # output rows per block
R = J + 2                    # 16 input rows per block
K = C * R                    # 128 contraction partitions
M = C * J                    # 112 output partitions
NBLK = 19                    # blocks per core
BLOCK_STARTS = [J * b for b in range(NBLK - 1)] + [ROWS_PER_CORE - J]
# h0(b) = 14*b for b<18, h0(18) = 242 (overlap-recompute tail)

COL_TILES = [(0, 512), (512, 512), (1024, 512), (1536, 510)]

import ml_dtypes

IN_DT = mybir.dt.float8e3    # on-wire activation dtype (e3m4: 1B, rel err 2^-5)
IN_NP = ml_dtypes.float8_e3m4
W_DT = mybir.dt.float16      # weights stay fp16 (exact); mixed-dtype matmul
W_NP = np.float16
OUT_DT = mybir.dt.float16    # on-wire output dtype (host upcasts)
OUT_NP = np.float16

Y_BUFS = 6
O_BUFS = 4
FIRST_CHUNKS = 2             # split the first y-load so MMs start earlier


def build_nc(repeat: int = 1, mode: str = "full", y_bufs: int = Y_BUFS,
             o_bufs: int = O_BUFS, first_chunks: int = FIRST_CHUNKS):
    do_mm = mode in ("full", "nocopy")
    do_copy = mode in ("full",)
    do_dma = mode in ("full", "nocopy", "dma")
    nc = bacc.Bacc(
        "TRN2",
        target_bir_lowering=False,
        debug=False,
        num_devices=N_CORES,
    )
    xp = nc.dram_tensor("xp", [K, NBLK, W], IN_DT, kind="ExternalInput").ap()
    wts = nc.dram_tensor("wts", [KW, K, M], W_DT, kind="ExternalInput").ap()
    op = nc.dram_tensor("op", [M, NBLK, W_OUT], OUT_DT, kind="ExternalOutput").ap()

    with tile.TileContext(nc) as tc:
        with (
            tc.tile_pool(name="wpool", bufs=1) as wpool,
            tc.tile_pool(name="ypool", bufs=y_bufs) as ypool,
            tc.tile_pool(name="opool", bufs=o_bufs) as opool,
            tc.tile_pool(name="pspool", bufs=8, space="PSUM") as pspool,
        ):
            wsb = wpool.tile([K, KW * M], W_DT)
            for kw in range(KW):
                nc.sync.dma_start(wsb[:, kw * M:(kw + 1) * M], wts[kw])

            for rep_i in range(repeat):
                for b in range(NBLK):
                    yt = ypool.tile([K, W], IN_DT, name="y", tag="y")
                    if do_dma:
                        if rep_i == 0 and b == 0 and first_chunks > 1:
                            cw = W // first_chunks
                            for c in range(first_chunks):
                                nc.gpsimd.dma_start(yt[:, c * cw:(c + 1) * cw],
                                                    xp[:, 0, c * cw:(c + 1) * cw])
                        else:
                            nc.gpsimd.dma_start(yt[:], xp[:, b:b + 1, :])

                    o = opool.tile([M, W_OUT], OUT_DT, name="o", tag="o")
                    pss = []
                    for ti in range(len(COL_TILES)):
                        ps = pspool.tile([M, 512], mybir.dt.float32,
                                         name=f"ps{ti}", tag="ps")
                        pss.append(ps)
                    if do_mm:
                        # kw-outer: consecutive MMs share the stationary side
                        for kw in range(KW):
                            for ti, (w0, n) in enumerate(COL_TILES):
                                nc.tensor.matmul(
                                    pss[ti][:, :n],
                                    lhsT=wsb[:, kw * M:(kw + 1) * M],
                                    rhs=yt[:, w0 + kw:w0 + kw + n],
                                    start=(kw == 0),
                                    stop=(kw == KW - 1),
                                )
                    if do_copy:
                        for ti, (w0, n) in enumerate(COL_TILES):
                            dst = o[:, w0:w0 + n]
                            if ti % 2 == 0:
                                nc.vector.tensor_copy(dst, pss[ti][:, :n])
                            else:
                                nc.scalar.copy(dst, pss[ti][:, :n])
                    if not do_copy and do_dma:
                        # ablation modes: cheap writer so Tile allocates o
                        nc.vector.memset(o[:, :8], 0.0)
                    if do_dma:
                        nc.sync.dma_start(op[:, b, :], o[:])

    nc.compile()
    return nc


def build_weight_lhst(weight: np.ndarray) -> np.ndarray:
    """weight: (C_out, C_in, 3, 3) fp32 -> (3, K, M) IN_NP."""
    wl = np.zeros((KW, K, M), np.float32)
    ci = np.arange(C)
    for kw in range(KW):
        for co in range(C):
            for j in range(J):
                for kh in range(KH):
                    r = j + kh
                    wl[kw, ci * R + r, co * J + j] = weight[co, :, kh, kw]
    return wl.astype(W_NP)


def pack_core_input(slab: np.ndarray) -> np.ndarray:
    """slab: (C, IN_ROWS, W) fp16 -> xp (K, NBLK, W) fp16."""
    s0, s1, s2 = slab.strides
    # b = 0..17 uniform stride J; b = 18 special (h0 = 242)
    v = np.lib.stride_tricks.as_strided(
        slab, shape=(C, R, NBLK - 1, W), strides=(s0, s1, J * s1, s2)
    )
    xp = np.empty((C, R, NBLK, W), slab.dtype)
    xp[:, :, :NBLK - 1, :] = v
    xp[:, :, NBLK - 1, :] = slab[:, BLOCK_STARTS[-1]:BLOCK_STARTS[-1] + R, :]
    return xp.reshape(K, NBLK, W)


def unpack_core_output(op: np.ndarray) -> np.ndarray:
    """op: (M, NBLK, W_OUT) -> (C, ROWS_PER_CORE, W_OUT) float32."""
    op = op.reshape(C, J, NBLK, W_OUT)
    res = np.empty((C, ROWS_PER_CORE, W_OUT), np.float32)
    res[:, BLOCK_STARTS[-1]:, :] = op[:, :, NBLK - 1, :].astype(np.float32)
    res[:, :J * (NBLK - 1), :] = (
        op[:, :, :NBLK - 1, :].transpose(0, 2, 1, 3).reshape(C, J * (NBLK - 1), W_OUT)
    )
    return res


def shard_inputs(x: np.ndarray, weight: np.ndarray):
    xc = np.ascontiguousarray(x).astype(IN_NP)
    wl = build_weight_lhst(weight)
    in_maps = []
    for i in range(N_CORES):
        lo = i * ROWS_PER_CORE
        hi = min(lo + IN_ROWS, H)
        if hi - lo == IN_ROWS:
            slab = xc[:, lo:hi, :]
        else:
            slab = np.zeros((C, IN_ROWS, W), IN_NP)
            slab[:, :hi - lo, :] = xc[:, lo:hi, :]
        in_maps.append({"xp": pack_core_input(slab), "wts": wl})
    return in_maps


def unshard_output(results) -> np.ndarray:
    parts = []
    for i in range(N_CORES):
        rows = ROWS_PER_CORE if i < N_CORES - 1 else H_OUT - (N_CORES - 1) * ROWS_PER_CORE
        parts.append(unpack_core_output(results[i]["op"])[:, :rows, :])
    return np.concatenate(parts, axis=1)


_NC_CACHE = None


def _get_nc():
    global _NC_CACHE
    if _NC_CACHE is None:
        _NC_CACHE = build_nc()
    return _NC_CACHE


def run(inputs: dict, **spmd_kwargs):
    """Run the conv on 8 NeuronCores. Returns (full_output, BassKernelResults)."""
    in_maps = shard_inputs(np.asarray(inputs["x"]), np.asarray(inputs["weight"]))
    nc = _get_nc()
    res = run_bass_kernel_spmd(nc, in_maps, core_ids=list(range(N_CORES)), **spmd_kwargs)
    return unshard_output(res.results).astype(np.float32), res


def kernel(**inputs) -> np.ndarray:
    out, _ = run(inputs)
    return out



# revision 8
# speedup vs baseline: 1.0517x; 1.0348x over previous
"""Trainium2 Bass kernel for 3x3 valid conv (C_in=8, C_out=8, H=W=2048).

Strategy (2-D spatial sharding, 4 H-bands x 2 W-halves across 8 cores):
  - Host splits the image into a 4x2 grid: per core ~512 output rows
    (+2 halo) x 1023 output cols (+2 halo), and packs each slab into the
    SBUF layout the TensorE wants:
        xp[(ci, r), b, w] = slab[ci, h0(b) + r, w]
    for 37 row-blocks b of J=14 output rows (h0 = 14*b, last block
    overlap-recomputes), r = 0..15. Halo rows are duplicated host-side so
    every device load is a plain contiguous slice.
  - On-wire dtypes: activations go over HBM as float8e3 (e3m4, 1 B/elem;
    measured end-to-end rel err 1.45e-2 vs the 2e-2 budget), weights stay
    fp16 and the TensorE runs mixed operand dtypes (fp16 lhsT x fp8e3 rhs
    -> fp32 PSUM, verified bit-accurate on HW). Outputs remain fp16.
  - Three lhsT weight matrices (one per kw) of shape [K=128, M=112]:
        K = (ci, r), M = (co, j), lhsT[kw][ci*16+r, co*14+j] =
        W[co, ci, r-j, kw] for 0 <= r-j <= 2.
  - Device per core: per block, one DMA loads [128, 1025] fp8 (first block
    split in two so the PE starts earlier); per column tile (512 + 511),
    3 accumulating matmuls (kw = 0,1,2; rhs shifted along the free axis)
    produce [112, <=512] fp32 PSUM; PSUM tiles are copied (DVE/ACT,
    fp32->fp16) and stored per block. Host scatters op back to
    (C, 2046, 2046) fp32.

  The schedule is PE-bound: the 4x2 grid needs 37*2*3 = 222
  ldweights+matmul pairs/core (vs 228 for 8x1) at ~244 ns each
  (~512 cols * 0.417 ns + ~30 ns fixed per-pair overhead) ~= 54 us, with
  DMA (~13 MB/core at ~360 GB/s ~= 36 us) fully overlapped. TimelineSim:
  56117 ns; measured in-situ HW matmul-loop rate agrees within a few %.
"""
import numpy as np
import ml_dtypes

import concourse.mybir as mybir
import concourse.tile as tile
from concourse import bacc
from concourse.bass_utils import run_bass_kernel_spmd

C = 8
H = W = 2048
KH = KW = 3
H_OUT = W_OUT = 2046
N_CORES = 8

HB = 4                 # H bands
WB = 2                 # W halves
BAND_ROWS = [512, 512, 512, 510]    # output rows per band (sum 2046)
COL_W = 1023           # output cols per W-half
WIN = COL_W + 2        # 1025 input cols per core

J = 14
R = 16
K = C * R              # 128
M = C * J              # 112
NBLK = 37              # ceil(512/14)
COL_TILES = [(0, 512), (512, 511)]

IN_DT = mybir.dt.float8e3
IN_NP = ml_dtypes.float8_e3m4
W_DT = mybir.dt.float16
W_NP = np.float16
OUT_DT = mybir.dt.float16

Y_BUFS = 6
O_BUFS = 4
FIRST_CHUNKS = 2


def block_starts(rows):
    return [J * b for b in range(NBLK - 1)] + [rows - J]


def build_nc(repeat: int = 1, mode: str = "full"):
    do_mm = mode in ("full", "nocopy")
    do_copy = mode in ("full",)
    do_dma = mode in ("full", "nocopy", "dma")
    nc = bacc.Bacc("TRN2", target_bir_lowering=False, debug=False,
                   num_devices=N_CORES)
    xp = nc.dram_tensor("xp", [K, NBLK, WIN], IN_DT, kind="ExternalInput").ap()
    wts = nc.dram_tensor("wts", [KW, K, M], W_DT, kind="ExternalInput").ap()
    op = nc.dram_tensor("op", [M, NBLK, COL_W], OUT_DT, kind="ExternalOutput").ap()

    with tile.TileContext(nc) as tc:
        with (
            tc.tile_pool(name="wpool", bufs=1) as wpool,
            tc.tile_pool(name="ypool", bufs=Y_BUFS) as ypool,
            tc.tile_pool(name="opool", bufs=O_BUFS) as opool,
            tc.tile_pool(name="pspool", bufs=8, space="PSUM") as pspool,
        ):
            wsb = wpool.tile([K, KW * M], W_DT)
            for kw in range(KW):
                nc.sync.dma_start(wsb[:, kw * M:(kw + 1) * M], wts[kw])

            for rep_i in range(repeat):
                for b in range(NBLK):
                    yt = ypool.tile([K, WIN], IN_DT, name="y", tag="y")
                    if do_dma:
                        if rep_i == 0 and b == 0 and FIRST_CHUNKS > 1:
                            cw = WIN // FIRST_CHUNKS
                            bounds = [0, cw, WIN]
                            for c in range(FIRST_CHUNKS):
                                nc.gpsimd.dma_start(
                                    yt[:, bounds[c]:bounds[c + 1]],
                                    xp[:, 0, bounds[c]:bounds[c + 1]])
                        else:
                            nc.gpsimd.dma_start(yt[:], xp[:, b:b + 1, :])

                    o = opool.tile([M, COL_W], OUT_DT, name="o", tag="o")
                    pss = [pspool.tile([M, 512], mybir.dt.float32,
                                       name=f"ps{ti}", tag="ps")
                           for ti in range(len(COL_TILES))]
                    if do_mm:
                        for kw in range(KW):
                            for ti, (w0, n) in enumerate(COL_TILES):
                                nc.tensor.matmul(
                                    pss[ti][:, :n],
                                    lhsT=wsb[:, kw * M:(kw + 1) * M],
                                    rhs=yt[:, w0 + kw:w0 + kw + n],
                                    start=(kw == 0),
                                    stop=(kw == KW - 1),
                                )
                    if do_copy:
                        for ti, (w0, n) in enumerate(COL_TILES):
                            dst = o[:, w0:w0 + n]
                            if ti % 2 == 0:
                                nc.vector.tensor_copy(dst, pss[ti][:, :n])
                            else:
                                nc.scalar.copy(dst, pss[ti][:, :n])
                    if not do_copy and do_dma:
                        nc.vector.memset(o[:, :8], 0.0)
                    if do_dma:
                        nc.sync.dma_start(op[:, b, :], o[:])

    nc.compile()
    return nc


def build_weight_lhst(weight: np.ndarray) -> np.ndarray:
    wl = np.zeros((KW, K, M), np.float32)
    for kw in range(KW):
        for co in range(C):
            for j in range(J):
                for kh in range(KH):
                    r = j + kh
                    wl[kw, np.arange(C) * R + r, co * J + j] = weight[co, :, kh, kw]
    return wl.astype(W_NP)


def pack_core_input(slab: np.ndarray, rows: int) -> np.ndarray:
    """slab: (C, rows+2, WIN) e3m4 -> xp (K, NBLK, WIN)."""
    s0, s1, s2 = slab.strides
    v = np.lib.stride_tricks.as_strided(
        slab, shape=(C, R, NBLK - 1, WIN), strides=(s0, s1, J * s1, s2))
    xp = np.empty((C, R, NBLK, WIN), slab.dtype)
    xp[:, :, :NBLK - 1, :] = v
    ls = rows - J
    xp[:, :, NBLK - 1, :] = slab[:, ls:ls + R, :]
    return xp.reshape(K, NBLK, WIN)


def unpack_core_output(op: np.ndarray, rows: int) -> np.ndarray:
    op = op.reshape(C, J, NBLK, COL_W)
    res = np.empty((C, rows, COL_W), np.float32)
    res[:, :J * (NBLK - 1), :] = (
        op[:, :, :NBLK - 1, :].transpose(0, 2, 1, 3).reshape(C, J * (NBLK - 1), COL_W))
    res[:, rows - J:, :] = op[:, :, NBLK - 1, :].astype(np.float32)
    return res


def shard_inputs(x: np.ndarray, weight: np.ndarray):
    xc = np.ascontiguousarray(x).astype(IN_NP)
    wl = build_weight_lhst(weight)
    in_maps = []
    for cid in range(N_CORES):
        hb, wh = cid // WB, cid % WB
        rows = BAND_ROWS[hb]
        rlo = sum(BAND_ROWS[:hb])
        clo = wh * COL_W
        slab = xc[:, rlo:rlo + rows + 2, clo:clo + WIN]
        in_maps.append({"xp": pack_core_input(slab, rows), "wts": wl})
    return in_maps


def unshard_output(results) -> np.ndarray:
    out = np.empty((C, H_OUT, W_OUT), np.float32)
    for cid in range(N_CORES):
        hb, wh = cid // WB, cid % WB
        rows = BAND_ROWS[hb]
        rlo = sum(BAND_ROWS[:hb])
        clo = wh * COL_W
        out[:, rlo:rlo + rows, clo:clo + COL_W] = \
            unpack_core_output(results[cid]["op"], rows)
    return out


_NC_CACHE = None


def _get_nc():
    global _NC_CACHE
    if _NC_CACHE is None:
        _NC_CACHE = build_nc()
    return _NC_CACHE


def run(inputs: dict, **spmd_kwargs):
    in_maps = shard_inputs(np.asarray(inputs["x"]), np.asarray(inputs["weight"]))
    nc = _get_nc()
    res = run_bass_kernel_spmd(nc, in_maps, core_ids=list(range(N_CORES)), **spmd_kwargs)
    return unshard_output(res.results).astype(np.float32), res


def kernel(**inputs) -> np.ndarray:
    out, _ = run(inputs)
    return out


# revision 10
# speedup vs baseline: 1.0531x; 1.0013x over previous
"""Trainium2 Bass kernel for 3x3 valid conv (C_in=8, C_out=8, H=W=2048).

Strategy (2-D spatial sharding, 4 H-bands x 2 W-halves across 8 cores):
  - Host splits the image into a 4x2 grid: per core ~512 output rows
    (+2 halo) x 1023 output cols (+2 halo), and packs each slab into the
    SBUF layout the TensorE wants:
        xp[(ci, r), b, w] = slab[ci, h0(b) + r, w]
    for 37 row-blocks b of J=14 output rows (h0 = 14*b, last block
    overlap-recomputes), r = 0..15. Halo rows are duplicated host-side so
    every device load is a plain contiguous slice.
  - On-wire dtypes: activations go over HBM as float8e3 (e3m4, 1 B/elem;
    measured end-to-end rel err 1.45e-2 vs the 2e-2 budget), weights stay
    fp16 and the TensorE runs mixed operand dtypes (fp16 lhsT x fp8e3 rhs
    -> fp32 PSUM, verified bit-accurate on HW). Outputs remain fp16.
  - Three lhsT weight matrices (one per kw) of shape [K=128, M=112]:
        K = (ci, r), M = (co, j), lhsT[kw][ci*16+r, co*14+j] =
        W[co, ci, r-j, kw] for 0 <= r-j <= 2.
  - Device per core: per block, one DMA loads [128, 1025] fp8 (first block
    split in two so the PE starts earlier); per column tile (512 + 511),
    3 accumulating matmuls (kw = 0,1,2; rhs shifted along the free axis)
    produce [112, <=512] fp32 PSUM; PSUM tiles are copied (DVE/ACT,
    fp32->fp16) and stored per block. Host scatters op back to
    (C, 2046, 2046) fp32.

  The schedule is PE-bound: the 4x2 grid needs 37*2*3 = 222
  ldweights+matmul pairs/core (vs 228 for 8x1) at ~244 ns each
  (~512 cols * 0.417 ns + ~30 ns fixed per-pair overhead) ~= 54 us, with
  DMA (~13 MB/core at ~360 GB/s ~= 36 us) fully overlapped. TimelineSim:
  56045 ns; measured in-situ HW matmul-loop rate agrees within a few %.
"""
import numpy as np
import ml_dtypes

import concourse.mybir as mybir
import concourse.tile as tile
from concourse import bacc
from concourse.bass_utils import run_bass_kernel_spmd

C = 8
H = W = 2048
KH = KW = 3
H_OUT = W_OUT = 2046
N_CORES = 8

HB = 4                 # H bands
WB = 2                 # W halves
BAND_ROWS = [512, 512, 512, 510]    # output rows per band (sum 2046)
COL_W = 1023           # output cols per W-half
WIN = COL_W + 2        # 1025 input cols per core

J = 14
R = 16
K = C * R              # 128
M = C * J              # 112
NBLK = 37              # ceil(512/14)
COL_TILES = [(0, 512), (512, 511)]

IN_DT = mybir.dt.float8e3
IN_NP = ml_dtypes.float8_e3m4
W_DT = mybir.dt.float16
W_NP = np.float16
OUT_DT = mybir.dt.float16

Y_BUFS = 6
O_BUFS = 4
FIRST_CHUNKS = 2


def block_starts(rows):
    return [J * b for b in range(NBLK - 1)] + [rows - J]


def build_nc(repeat: int = 1, mode: str = "full"):
    do_mm = mode in ("full", "nocopy")
    do_copy = mode in ("full",)
    do_dma = mode in ("full", "nocopy", "dma")
    nc = bacc.Bacc("TRN2", target_bir_lowering=False, debug=False,
                   num_devices=N_CORES)
    xp = nc.dram_tensor("xp", [K, NBLK, WIN], IN_DT, kind="ExternalInput").ap()
    wts = nc.dram_tensor("wts", [KW, K, M], W_DT, kind="ExternalInput").ap()
    op = nc.dram_tensor("op", [M, NBLK, COL_W], OUT_DT, kind="ExternalOutput").ap()

    with tile.TileContext(nc) as tc:
        with (
            tc.tile_pool(name="wpool", bufs=1) as wpool,
            tc.tile_pool(name="ypool", bufs=Y_BUFS) as ypool,
            tc.tile_pool(name="opool", bufs=O_BUFS) as opool,
            tc.tile_pool(name="pspool", bufs=8, space="PSUM") as pspool,
        ):
            wsb = wpool.tile([K, KW * M], W_DT)
            for kw in range(KW):
                nc.sync.dma_start(wsb[:, kw * M:(kw + 1) * M], wts[kw])

            for rep_i in range(repeat):
                for b in range(NBLK):
                    yt = ypool.tile([K, WIN], IN_DT, name="y", tag="y")
                    if do_dma:
                        if rep_i == 0 and b == 0 and FIRST_CHUNKS > 1:
                            # split at 516 so the tile-0 chains (cols 0..514
                            # for kw<=2) depend only on the first chunk
                            bounds = [0, 516, WIN]
                            for c in range(FIRST_CHUNKS):
                                nc.gpsimd.dma_start(
                                    yt[:, bounds[c]:bounds[c + 1]],
                                    xp[:, 0, bounds[c]:bounds[c + 1]])
                        else:
                            nc.gpsimd.dma_start(yt[:], xp[:, b:b + 1, :])

                    o = opool.tile([M, COL_W], OUT_DT, name="o", tag="o")
                    pss = [pspool.tile([M, 512], mybir.dt.float32,
                                       name=f"ps{ti}", tag="ps")
                           for ti in range(len(COL_TILES))]
                    if do_mm:
                        for kw in range(KW):
                            for ti, (w0, n) in enumerate(COL_TILES):
                                nc.tensor.matmul(
                                    pss[ti][:, :n],
                                    lhsT=wsb[:, kw * M:(kw + 1) * M],
                                    rhs=yt[:, w0 + kw:w0 + kw + n],
                                    start=(kw == 0),
                                    stop=(kw == KW - 1),
                                )
                    if do_copy:
                        for ti, (w0, n) in enumerate(COL_TILES):
                            dst = o[:, w0:w0 + n]
                            if ti % 2 == 0:
                                nc.vector.tensor_copy(dst, pss[ti][:, :n])
                            else:
                                nc.scalar.copy(dst, pss[ti][:, :n])
                    if not do_copy and do_dma:
                        nc.vector.memset(o[:, :8], 0.0)
                    if do_dma:
                        nc.sync.dma_start(op[:, b, :], o[:])

    nc.compile()
    return nc


def build_weight_lhst(weight: np.ndarray) -> np.ndarray:
    wl = np.zeros((KW, K, M), np.float32)
    for kw in range(KW):
        for co in range(C):
            for j in range(J):
                for kh in range(KH):
                    r = j + kh
                    wl[kw, np.arange(C) * R + r, co * J + j] = weight[co, :, kh, kw]
    return wl.astype(W_NP)


def pack_core_input(slab: np.ndarray, rows: int) -> np.ndarray:
    """slab: (C, rows+2, WIN) e3m4 -> xp (K, NBLK, WIN)."""
    s0, s1, s2 = slab.strides
    v = np.lib.stride_tricks.as_strided(
        slab, shape=(C, R, NBLK - 1, WIN), strides=(s0, s1, J * s1, s2))
    xp = np.empty((C, R, NBLK, WIN), slab.dtype)
    xp[:, :, :NBLK - 1, :] = v
    ls = rows - J
    xp[:, :, NBLK - 1, :] = slab[:, ls:ls + R, :]
    return xp.reshape(K, NBLK, WIN)


def unpack_core_output(op: np.ndarray, rows: int) -> np.ndarray:
    op = op.reshape(C, J, NBLK, COL_W)
    res = np.empty((C, rows, COL_W), np.float32)
    res[:, :J * (NBLK - 1), :] = (
        op[:, :, :NBLK - 1, :].transpose(0, 2, 1, 3).reshape(C, J * (NBLK - 1), COL_W))
    res[:, rows - J:, :] = op[:, :, NBLK - 1, :].astype(np.float32)
    return res


def shard_inputs(x: np.ndarray, weight: np.ndarray):
    xc = np.ascontiguousarray(x).astype(IN_NP)
    wl = build_weight_lhst(weight)
    in_maps = []
    for cid in range(N_CORES):
        hb, wh = cid // WB, cid % WB
        rows = BAND_ROWS[hb]
        rlo = sum(BAND_ROWS[:hb])
        clo = wh * COL_W
        slab = xc[:, rlo:rlo + rows + 2, clo:clo + WIN]
        in_maps.append({"xp": pack_core_input(slab, rows), "wts": wl})
    return in_maps


def unshard_output(results) -> np.ndarray:
    out = np.empty((C, H_OUT, W_OUT), np.float32)
    for cid in range(N_CORES):
        hb, wh = cid // WB, cid % WB
        rows = BAND_ROWS[hb]
        rlo = sum(BAND_ROWS[:hb])
        clo = wh * COL_W
        out[:, rlo:rlo + rows, clo:clo + COL_W] = \
            unpack_core_output(results[cid]["op"], rows)
    return out


_NC_CACHE = None


def _get_nc():
    global _NC_CACHE
    if _NC_CACHE is None:
        _NC_CACHE = build_nc()
    return _NC_CACHE


def run(inputs: dict, **spmd_kwargs):
    in_maps = shard_inputs(np.asarray(inputs["x"]), np.asarray(inputs["weight"]))
    nc = _get_nc()
    res = run_bass_kernel_spmd(nc, in_maps, core_ids=list(range(N_CORES)), **spmd_kwargs)
    return unshard_output(res.results).astype(np.float32), res


def kernel(**inputs) -> np.ndarray:
    out, _ = run(inputs)
    return out
